# revision 2
# baseline (speedup 1.0000x reference)
"""Trainium2 Bass kernel for nn_DQN_34136400069239 (DeepSets-style pooling).

Math (reference):
    h1  = relu(x @ pw1 + pb1)          [N, H]
    h2  = relu(h1 @ pw2 + pb2)         [N, H]
    phi = h2 @ pw3 + pb3               [N, F]
    fp  = sum(phi, axis=0)             [F]
    ... tiny rho MLP + concat(x_static) + tiny 3-layer MLP -> [OUT]

The third phi layer is linear, so fp = (sum_n h2[n]) @ pw3 + N*pb3 and the
device only computes S = sum_n relu(h1 @ pw2 + pb2); the tiny tail runs on
host in float64.

Device strategy: data-parallel over rows, 8 cores x 50000 rows. Per core the
kernel streams 500/512-row blocks:
  PE : layer-1 fp16 matmuls (bias via a ones-row; h1 pre-scaled by 16 via W1)
       + layer-2 matmuls in fp8-e4m3 DoubleRow perf mode (4x MAC rate vs
       fp16). W2 is sent as hi+lo e4m3 pair (scaled by 1024), making the
       layer-2 weights effectively exact; only the h1 e4m3 rounding remains.
  ACT/DVE: relu1 (PSUM->SBUF fp8 cast) and relu2+row-sum. These two engines
       are the throughput bound: they are the only engines that can read
       PSUM (GPSIMD has no PSUM port, DMA cannot read PSUM).

Modes (DQN_MODE env):
  f16  fp16 everywhere; relu1 on DVE (one [128,2,500] op), relu2 on ACT
       (2 biased-relu ops with accum_out).               <- default
  drp  layer-2 in fp8 DoubleRow, emitted in block PAIRS so each stationary
       weight set serves two back-to-back matmuls and the dual-fp8 weight
       loads hide behind compute.
  tr   transposed layer 2 (rows on PSUM partitions via h1-as-stationary
       DoubleRow): relu1 on ACT (2 ops), relu2 as ONE DVE max-tensor op, and
       the row-sums via accumulating ones-matmuls on the PE - no ACT
       accumulator reads at all. Needs 512-row blocks (dual-fp8 LDW
       alignment), rows padded to 50176/core, pad contribution subtracted
       exactly on host.

All three modes are bound by the same wall: ACT+DVE are the only engines
that can read PSUM, and the 4 relu column-streams per block saturate them
(~930ns/block theoretical, ~1200-1500 with per-instruction overheads).
fp8 matmuls relieve the PE but cannot move that wall, so they do not help
end-to-end; they are kept for hardware where the vector side is faster.

Measured (slope method, K=49, all 8 cores over 400k rows; per-session
clock variance is +-15%, interleaved medians over 12+ runs):
  f16 ~132 us (min 110, rel err 2.6e-4)   <- default
  drp ~139-149 us (min 114, rel err 7.7e-4)
  tr  ~144 us (min 132, rel err 8.4e-4)
"""

import os

import numpy as np

N = 400000
IN, H, F, S_STATIC, OUT = 64, 256, 128, 16, 5
N_CORES = 8
R = N // N_CORES          # rows per core
BLK = 500                 # block rows (f16/drp modes)
NBLK = R // BLK
TR_BLK = 512              # block rows (tr mode; dual-fp8 LDW alignment)
TR_R = 50176              # R padded to a multiple of TR_BLK

SC1 = 16.0                # h1 scale (folded into W1; exact power of 2)
SC2 = 1024.0              # W2 scale (exact power of 2)

MODE = os.environ.get("DQN_MODE", "f16")

_prog_cache: dict = {}


def _build(mode: str, iters: int = 1, r: int = R, xbufs: int = 4, hbufs: int = 4):
    import concourse.mybir as mybir
    import concourse.tile as tile
    from concourse import bacc
    from contextlib import ExitStack

    dt = mybir.dt
    f32 = dt.float32
    f16 = dt.float16
    e4 = dt.float8e4
    bf16 = dt.bfloat16
    h1_dt = f16 if mode == "f16" else e4
    Relu = mybir.ActivationFunctionType.Relu
    X = mybir.AxisListType.X
    DR = mybir.MatmulPerfMode.DoubleRow

    if mode == "tr" and r % TR_BLK != 0:
        r = (r + TR_BLK - 1) // TR_BLK * TR_BLK

    nc = bacc.Bacc("TRN2", target_bir_lowering=False, debug=False,
                   enable_asserts=False, num_devices=1)

    d_xt = nc.dram_tensor("d_xt", [IN + 1, r], f16, kind="ExternalInput").ap()
    d_w1 = nc.dram_tensor("d_w1", [IN + 1, H], f16, kind="ExternalInput").ap()
    if mode == "f16":
        d_w2 = nc.dram_tensor("d_w2", [H, H], f16, kind="ExternalInput").ap()
    elif mode == "tr":
        d_w2hm = nc.dram_tensor("d_w2hm", [128, 2, H], e4, kind="ExternalInput").ap()
        d_w2lm = nc.dram_tensor("d_w2lm", [128, 2, H], e4, kind="ExternalInput").ap()
        d_b2t = nc.dram_tensor("d_b2t", [128, 4, H], f32, kind="ExternalInput").ap()
        d_ones = nc.dram_tensor("d_ones", [128, 1], bf16, kind="ExternalInput").ap()
    else:
        d_w2h = nc.dram_tensor("d_w2h", [128, 2, 2, 128], e4, kind="ExternalInput").ap()
        d_w2l = nc.dram_tensor("d_w2l", [128, 2, 2, 128], e4, kind="ExternalInput").ap()
    if mode != "tr":
        d_b2p = nc.dram_tensor("d_b2p", [128, 2], f32, kind="ExternalInput").ap()
    if mode == "tr":
        d_s = nc.dram_tensor("d_s", [1, H], f32, kind="ExternalOutput").ap()
    else:
        d_s = nc.dram_tensor("d_s", [128, 2], f32, kind="ExternalOutput").ap()

    with tile.TileContext(nc) as tc, ExitStack() as ctx:
        cpool = ctx.enter_context(tc.tile_pool(name="cpool", bufs=1))
        xpool = ctx.enter_context(tc.tile_pool(name="xpool", bufs=xbufs))
        hpool = ctx.enter_context(tc.tile_pool(name="hpool", bufs=hbufs))
        spool = ctx.enter_context(tc.tile_pool(name="spool", bufs=3))
        if mode == "tr":
            ps1p = ctx.enter_context(tc.tile_pool(name="ps1p", bufs=3, space="PSUM"))
            ps2p = ctx.enter_context(tc.tile_pool(name="ps2p", bufs=2, space="PSUM"))
            sump = ctx.enter_context(tc.tile_pool(name="sump", bufs=1, space="PSUM"))
        else:
            ps1p = ctx.enter_context(tc.tile_pool(name="ps1p", bufs=2, space="PSUM"))
            ps2p = ctx.enter_context(tc.tile_pool(
                name="ps2p", bufs=1 if mode == "drp" else 2, space="PSUM"))

        w1_sb = cpool.tile([IN + 1, H], f16, name="w1_sb")
        nc.sync.dma_start(w1_sb[:], d_w1)

        if mode == "f16":
            w2_sb = []
            for k in range(2):
                t = cpool.tile([128, H], f16, name=f"w2_sb{k}")
                nc.sync.dma_start(t[:], d_w2[k * 128:(k + 1) * 128, :])
                w2_sb.append(t)
        elif mode == "tr":
            w2hm_sb = cpool.tile([128, 2, H], e4, name="w2hm_sb")
            nc.sync.dma_start(w2hm_sb[:], d_w2hm)
            w2lm_sb = cpool.tile([128, 2, H], e4, name="w2lm_sb")
            nc.sync.dma_start(w2lm_sb[:], d_w2lm)
            b2t_sb = cpool.tile([128, 4, H], f32, name="b2t_sb")
            nc.sync.dma_start(b2t_sb[:], d_b2t)
            ones_sb = cpool.tile([128, 1], bf16, name="ones_sb")
            nc.sync.dma_start(ones_sb[:], d_ones)
        else:
            w2h_sb = cpool.tile([128, 2, 2, 128], e4, name="w2h_sb")
            nc.sync.dma_start(w2h_sb[:], d_w2h)
            w2l_sb = cpool.tile([128, 2, 2, 128], e4, name="w2l_sb")
            nc.sync.dma_start(w2l_sb[:], d_w2l)
        if mode != "tr":
            b2p_sb = cpool.tile([128, 2], f32, name="b2p_sb")
            nc.sync.dma_start(b2p_sb[:], d_b2p)

        if mode == "tr":
            CH = TR_BLK // 4
            nblk = r // TR_BLK
            sum_ps = sump.tile([128, 512], f32, name="sum_ps")
            blocks = [b for _ in range(iters) for b in range(nblk)]
            nb_tot = len(blocks)
            for i, b in enumerate(blocks):
                xt = xpool.tile([IN + 1, TR_BLK], f16, name="xt", tag="xt")
                nc.sync.dma_start(xt[:], d_xt[:, b * TR_BLK:(b + 1) * TR_BLK])

                h1 = hpool.tile([128, 2, TR_BLK], e4, name="h1", tag="h1")
                for m in range(2):
                    ps1 = ps1p.tile([128, 512], f32, name="ps1", tag="ps1")
                    nc.tensor.matmul(
                        ps1[:, 0:TR_BLK],
                        w1_sb[:, m * 128:(m + 1) * 128],
                        xt[:], start=True, stop=True,
                    )
                    nc.scalar.activation(h1[:, m, :], ps1[:, 0:TR_BLK], Relu)

                ps2t = ps2p.tile([128, 4, H], f32, name="ps2t", tag="ps2t")
                for c in range(4):
                    lhsT = h1[:, :, c * CH:(c + 1) * CH]
                    nc.tensor.matmul(ps2t[0:CH, c, :], lhsT, w2hm_sb[:],
                                     start=True, stop=False, perf_mode=DR)
                    nc.tensor.matmul(ps2t[0:CH, c, :], lhsT, w2lm_sb[:],
                                     start=False, stop=True, perf_mode=DR)

                scrt = spool.tile([128, 4, H], bf16, name="scrt", tag="scrt")
                nc.vector.tensor_tensor(
                    scrt[0:CH, :, :], ps2t[0:CH, :, :], b2t_sb[0:CH, :, :],
                    op=mybir.AluOpType.max,
                )

                for c in range(4):
                    nc.tensor.matmul(
                        sum_ps[0:1, 0:H], ones_sb[0:CH, :], scrt[0:CH, c, :],
                        start=(i == 0 and c == 0),
                        stop=(i == nb_tot - 1 and c == 3),
                    )

            s_sb = cpool.tile([1, H], f32, name="s_sb")
            nc.vector.tensor_copy(s_sb[:], sum_ps[0:1, 0:H])
            nc.sync.dma_start(d_s, s_sb[:])
        else:
            nblk = r // BLK
            acc = cpool.tile([128, 2, nblk], f32, name="acc")

            def emit_front(b):
                xt = xpool.tile([IN + 1, BLK], f16, name="xt", tag="xt")
                nc.sync.dma_start(xt[:], d_xt[:, b * BLK:(b + 1) * BLK])
                ps1 = ps1p.tile([128, 2, 512], f32, name="ps1", tag="ps1")
                for m in range(2):
                    nc.tensor.matmul(
                        ps1[:, m, 0:BLK],
                        w1_sb[:, m * 128:(m + 1) * 128],
                        xt[:], start=True, stop=True,
                    )
                h1 = hpool.tile([128, 2, BLK], h1_dt, name="h1", tag="h1")
                nc.vector.tensor_scalar_max(h1[:], ps1[:, :, 0:BLK], 0.0)
                return h1

            def emit_relu2(ps2, b):
                for hh in range(2):
                    scr = spool.tile([128, BLK], f32, name=f"scr{hh}",
                                     tag=f"scr{hh}")
                    nc.scalar.activation(
                        scr[:], ps2[:, hh, 0:BLK], Relu,
                        bias=b2p_sb[:, hh:hh + 1],
                        accum_out=acc[:, hh, b:b + 1],
                    )

            blocks = [b for _ in range(iters) for b in range(nblk)]
            if mode == "drp":
                for b0, b1 in zip(blocks[0::2], blocks[1::2]):
                    h1a = emit_front(b0)
                    h1b = emit_front(b1)
                    ps2a = ps2p.tile([128, 2, 512], f32, name="ps2", tag="ps2")
                    ps2b = ps2p.tile([128, 2, 512], f32, name="ps2b", tag="ps2b")
                    for m in range(2):
                        for w in (w2h_sb, w2l_sb):
                            first = w is w2h_sb
                            for ps2, h1 in ((ps2a, h1a), (ps2b, h1b)):
                                nc.tensor.matmul(
                                    ps2[:, m, 0:BLK], w[:, :, m, :], h1[:],
                                    start=first, stop=not first, perf_mode=DR,
                                )
                    emit_relu2(ps2a, b0)
                    emit_relu2(ps2b, b1)
            else:
                for b in blocks:
                    h1 = emit_front(b)
                    ps2 = ps2p.tile([128, 2, 512], f32, name="ps2", tag="ps2")
                    for m in range(2):
                        if mode == "f16":
                            for k in range(2):
                                nc.tensor.matmul(
                                    ps2[:, m, 0:BLK],
                                    w2_sb[k][:, m * 128:(m + 1) * 128],
                                    h1[:, k, :], start=(k == 0), stop=(k == 1),
                                )
                        else:
                            nc.tensor.matmul(
                                ps2[:, m, 0:BLK], w2h_sb[:, :, m, :], h1[:],
                                start=True, stop=False, perf_mode=DR)
                            nc.tensor.matmul(
                                ps2[:, m, 0:BLK], w2l_sb[:, :, m, :], h1[:],
                                start=False, stop=True, perf_mode=DR)
                    emit_relu2(ps2, b)

            s_sb = cpool.tile([128, 2], f32, name="s_sb")
            nc.vector.reduce_sum(s_sb[:], acc[:], axis=X)
            nc.sync.dma_start(d_s, s_sb[:])

    nc.compile()
    return nc


def _quant_w2(pw2):
    import ml_dtypes

    w2s = (SC2 * np.asarray(pw2, np.float32)).astype(np.float32)
    w2hi = w2s.astype(ml_dtypes.float8_e4m3)
    w2lo = (w2s - w2hi.astype(np.float32)).astype(ml_dtypes.float8_e4m3)
    return w2hi, w2lo


def _prep_in_maps(inputs: dict, mode: str, n_cores: int = N_CORES, r: int = R):
    import ml_dtypes

    x = np.asarray(inputs["x"], dtype=np.float32)
    pw1 = np.asarray(inputs["pw1"], dtype=np.float32)
    pb1 = np.asarray(inputs["pb1"], dtype=np.float32)
    pw2 = np.asarray(inputs["pw2"], dtype=np.float32)
    pb2 = np.asarray(inputs["pb2"], dtype=np.float32)

    sc1 = 1.0 if mode == "f16" else SC1
    w1_aug = (sc1 * np.concatenate([pw1, pb1[None, :]], axis=0)).astype(np.float16)
    com = {"d_w1": w1_aug}

    if mode == "f16":
        com["d_w2"] = pw2.astype(np.float16)
        sc = 1.0
    else:
        sc = SC1 * SC2
        w2hi, w2lo = _quant_w2(pw2)
        if mode == "tr":
            def mov(w):
                return np.ascontiguousarray(
                    w.reshape(2, 128, H).transpose(1, 0, 2))
            com["d_w2hm"] = mov(w2hi)
            com["d_w2lm"] = mov(w2lo)
            com["d_b2t"] = np.broadcast_to(
                (-sc * pb2).astype(np.float32), (128, 4, H)).copy()
            com["d_ones"] = np.ones((128, 1), ml_dtypes.bfloat16)
        else:
            def drl(w):
                return np.ascontiguousarray(
                    w.reshape(2, 128, 2, 128).transpose(1, 0, 2, 3))
            com["d_w2h"] = drl(w2hi)
            com["d_w2l"] = drl(w2lo)
    if mode != "tr":
        b2s = (sc * pb2).astype(np.float32).reshape(2, 128).T
        com["d_b2p"] = np.ascontiguousarray(b2s)

    r_eff = r
    if mode == "tr" and r_eff % TR_BLK != 0:
        r_eff = (r_eff + TR_BLK - 1) // TR_BLK * TR_BLK
    in_maps = []
    for c in range(n_cores):
        xt = np.zeros((IN + 1, r_eff), np.float16)
        xt[:IN, :r] = x[c * r:(c + 1) * r].T.astype(np.float16)
        xt[IN] = 1.0
        m = dict(com)
        m["d_xt"] = xt
        in_maps.append(m)
    return in_maps


def _gather_S(results, inputs: dict, mode: str, n_rows_total: int):
    """Combine per-core d_s into S = sum_n relu(h1@W2 + b2) (unscaled)."""
    import ml_dtypes

    pb2 = np.asarray(inputs["pb2"], dtype=np.float64)
    sc = 1.0 if mode == "f16" else SC1 * SC2
    if mode == "tr":
        S = np.zeros((1, H), np.float64)
        for rmap in results:
            S += rmap["d_s"].astype(np.float64)
        n_cores = len(results)
        r_real = n_rows_total // n_cores
        r_eff = (r_real + TR_BLK - 1) // TR_BLK * TR_BLK
        n_pad = (r_eff - r_real) * n_cores
        if n_pad:
            # pad rows (x=0) contribute exactly relu-chain(b1) through the
            # same quantized path; subtract that deterministic value
            pw1 = np.asarray(inputs["pw1"], dtype=np.float64)
            pb1 = np.asarray(inputs["pb1"], dtype=np.float64)
            w1a = (SC1 * np.concatenate([pw1, pb1[None, :]])).astype(np.float16)
            ps1_pad = w1a.astype(np.float64)[IN]
            h1_pad = np.maximum(ps1_pad, 0.0).astype(ml_dtypes.float8_e4m3)
            w2hi, w2lo = _quant_w2(inputs["pw2"])
            w2q = w2hi.astype(np.float64) + w2lo.astype(np.float64)
            z_pad = h1_pad.astype(np.float64) @ w2q
            m_pad = np.maximum(z_pad, -sc * pb2)
            S[0] -= n_pad * m_pad
        return S[0] / sc + n_rows_total * pb2
    # non-tr: ACT computed relu(z + sc*b2) directly -> no bias correction
    S = np.zeros((128, 2), np.float64)
    for rmap in results:
        S += rmap["d_s"].astype(np.float64)
    return (S / sc).T.reshape(H)


def _host_tail(S: np.ndarray, inputs: dict) -> np.ndarray:
    f = np.float64

    def g(name):
        return np.asarray(inputs[name], dtype=f)

    phi_sum = S @ g("pw3") + N * g("pb3")
    r = np.maximum(phi_sum @ g("rw1") + g("rb1"), 0.0)
    r = np.maximum(r @ g("rw2") + g("rb2"), 0.0)
    r = r @ g("rw3") + g("rb3")
    v = np.concatenate([r, g("x_static")])
    v = np.maximum(v @ g("w1") + g("b1"), 0.0)
    v = np.maximum(v @ g("w2") + g("b2"), 0.0)
    return (v @ g("w3") + g("b3")).astype(np.float32)


def _run(inputs: dict, mode: str | None = None):
    from concourse.bass_utils import run_bass_kernel_spmd

    mode = mode or MODE
    nc = _prog_cache.get(mode)
    if nc is None:
        nc = _build(mode)
        _prog_cache[mode] = nc

    in_maps = _prep_in_maps(inputs, mode)
    res = run_bass_kernel_spmd(
        nc, in_maps, core_ids=list(range(N_CORES)), trace=False,
    )
    S = _gather_S(res.results, inputs, mode, n_rows_total=N)
    return _host_tail(S, inputs), res


def kernel(**inputs) -> np.ndarray:
    out, _ = _run(inputs)
    return out
